# revision 3
# baseline (speedup 1.0000x reference)
"""MoE (noisy top-2 routing, 8 experts) on 8 Trainium2 NeuronCores.

Strategy (expert-parallel, per the sharding hint):
  - Router runs on host in float64 (134 MFLOP — negligible next to the
    137 GFLOP expert MLPs; fp64 makes the top-k selection robust).
  - Tokens are dispatched by top-2 expert id; core e gets expert e's
    weights and its routed tokens (padded to a fixed capacity).
  - Each core computes o = relu(x @ W1[e] + b1[e]) @ W2[e] for its
    tokens with float32r matmuls (full PE rate, ~tf32 accuracy).
  - Host combines: out[t] = g1*(o_e1[t] + b2[e1]) + g2*(o_e2[t] + b2[e2]).

Layout notes: activations are kept feature-major on device (xT [D, C],
hT [F, C]) so both matmul layers contract along the partition dim with
zero on-device transposes; the host supplies xT and receives o [C, D].
"""

import numpy as np

import concourse.bass as bass
import concourse.mybir as mybir
import concourse.tile as tile
from concourse import bacc
from concourse.bass import ts
from concourse.bass_utils import run_bass_kernel_spmd
from concourse.kernels.tile_matmul import (
    dma_from_dram_kxm,
    dma_from_dram_kxn,
    dma_to_dram_mxn,
    composable_matmul_tile_kernel,
    matmul_tile_kernel,
    k_pool_min_bufs,
)

B, S, D, F, E = 2, 2048, 1024, 4096, 8
T = B * S
TOP_K = 2
N_CORES = 8
CAP = 1536  # tokens per expert, padded; key(0) inputs max out at 1064

_build_cache = {}
_last_run = None


def profile_last(trace_cores=None):
    """Re-run the most recent kernel() dispatch with NTFF tracing; returns
    BassKernelResults (exec_time_ns etc.). Dev-harness helper only."""
    nc, in_maps = _last_run
    return run_bass_kernel_spmd(nc, in_maps, list(range(N_CORES)),
                                trace=True, trace_cores=trace_cores)


def _build(cap):
    """Compile the per-core expert-MLP kernel (SPMD: same program, per-core
    weights/tokens). Returns the compiled Bacc module."""
    if cap in _build_cache:
        return _build_cache[cap]

    f32 = mybir.dt.float32
    f32r = mybir.dt.float32r

    nc = bacc.Bacc("TRN2", target_bir_lowering=False, debug=False,
                   num_devices=N_CORES)
    xT = nc.dram_tensor("xT", [D, cap], f32r, kind="ExternalInput")
    w1 = nc.dram_tensor("w1", [D, F], f32r, kind="ExternalInput")
    b1 = nc.dram_tensor("b1", [128, F // 128], f32, kind="ExternalInput")
    w2 = nc.dram_tensor("w2", [F, D], f32r, kind="ExternalInput")
    hT = nc.dram_tensor("hT", [F, cap], f32r)
    o = nc.dram_tensor("o", [cap, D], f32, kind="ExternalOutput")

    with tile.TileContext(nc) as tc:
        from contextlib import ExitStack
        with ExitStack() as ctx:
            num_bufs = k_pool_min_bufs(xT.ap())
            kxm_pool = ctx.enter_context(tc.tile_pool(name="l1_kxm", bufs=num_bufs))
            kxn_pool = ctx.enter_context(tc.tile_pool(name="l1_kxn", bufs=num_bufs))
            const = ctx.enter_context(tc.tile_pool(name="const", bufs=1))

            b1_tile = const.tile([128, F // 128], f32)
            nc.sync.dma_start(b1_tile[:], b1.ap())

            def bias_relu(nc_, psum, sbuf, md):
                col = (md.m_tile_idx * md.m_tile) // 128 + md.m_subtile_idx
                nc_.scalar.activation(
                    sbuf[:], psum[:],
                    mybir.ActivationFunctionType.Relu,
                    bias=b1_tile[:, col:col + 1],
                )

            kxm_producer, kxm_shape = dma_from_dram_kxm(kxm_pool, w1.ap())
            kxn_producer, kxn_shape = dma_from_dram_kxn(kxn_pool, xT.ap())
            composable_matmul_tile_kernel(
                tc=tc,
                kxm_shape=kxm_shape,
                kxn_shape=kxn_shape,
                output_type=f32r,
                kxm_producer=kxm_producer,
                kxn_producer=kxn_producer,
                mxn_subtile_reducer=bias_relu,
                mxn_consumer=dma_to_dram_mxn(hT.ap()),
            )

        matmul_tile_kernel(tc, hT.ap(), w2.ap(), o.ap())

    nc.compile()
    _build_cache[cap] = nc
    return nc


def _route(x2d, noise2d, Wr, br, Wn, bn):
    """Noisy top-2 router in float64. Returns (top2 ids [T,2], gates [T,2])."""
    x64 = x2d.astype(np.float64)
    logits = x64 @ Wr.astype(np.float64) + br.astype(np.float64)
    nl = x64 @ Wn.astype(np.float64) + bn.astype(np.float64)
    noisy = logits + noise2d.astype(np.float64) * np.logaddexp(0.0, nl)
    # stable argsort of -noisy == jax.lax.top_k tie-breaking (lower index wins)
    top2 = np.argsort(-noisy, axis=-1, kind="stable")[:, :TOP_K]
    v = np.take_along_axis(noisy, top2, axis=-1)
    v = v - v.max(axis=-1, keepdims=True)
    ev = np.exp(v)
    gates = ev / ev.sum(axis=-1, keepdims=True)
    return top2, gates.astype(np.float64)


def kernel(x, noise, Wr, br, Wn, bn, W1, b1, W2, b2):
    x = np.ascontiguousarray(np.asarray(x, dtype=np.float32))
    x2d = x.reshape(T, D)
    top2, gates = _route(x2d, np.asarray(noise).reshape(T, E),
                         np.asarray(Wr), np.asarray(br),
                         np.asarray(Wn), np.asarray(bn))

    # dispatch: stable sort of the 2T assignments by expert id
    expert_ids = top2.ravel()  # assignment a -> expert; token = a // 2
    ord_ = np.argsort(expert_ids, kind="stable")
    counts = np.bincount(expert_ids, minlength=E)
    starts = np.zeros(E + 1, dtype=np.int64)
    np.cumsum(counts, out=starts[1:])

    cap = CAP
    while counts.max() > cap:
        cap += 512
    nc = _build(cap)

    W1 = np.asarray(W1, dtype=np.float32)
    W2 = np.asarray(W2, dtype=np.float32)
    b1 = np.asarray(b1, dtype=np.float32)
    b2 = np.asarray(b2, dtype=np.float32)

    in_maps = []
    tok_per_expert = []
    for e in range(E):
        toks = ord_[starts[e]:starts[e + 1]] // 2
        tok_per_expert.append(toks)
        xe = np.zeros((cap, D), dtype=np.float32)
        xe[:len(toks)] = x2d[toks]
        in_maps.append({
            "xT": np.ascontiguousarray(xe.T),
            "w1": W1[e],
            "b1": np.ascontiguousarray(b1[e].reshape(F // 128, 128).T),
            "w2": W2[e],
        })

    res = run_bass_kernel_spmd(nc, in_maps, list(range(N_CORES)))
    global _last_run
    _last_run = (nc, in_maps)

    # combine: A holds expert outputs in assignment-sorted order
    A = np.empty((2 * T, D), dtype=np.float32)
    pos = np.empty(2 * T, dtype=np.int64)
    pos[ord_] = np.arange(2 * T)
    for e in range(E):
        n_e = counts[e]
        A[starts[e]:starts[e + 1]] = res.results[e]["o"][:n_e] + b2[e]
    out = (gates[:, :, None].astype(np.float32)
           * A[pos.reshape(T, TOP_K)]).sum(axis=1)
    return out.reshape(B, S, D).astype(np.float32)
